# revision 9
# baseline (speedup 1.0000x reference)
# Trainium2 Bass kernel for nn_AttentionPropagation (SuperGlue-style bidirectional
# attentional propagation): 6x (1x1conv+BN+ReLU) filters + QK attention with
# softmax over BOTH axes + two aggregations + output filters.
#
# Sharding: 16 (batch, head) units over 8 cores -> each core owns batch b=core//2
# and a contiguous 128-channel (2-head) slice of the filter outputs.  The final
# filters f4/f5 need all 4 heads of a batch, so core pairs {2b, 2b+1} exchange
# their normalized 128-channel halves of add0/add1 with an AllGather, then each
# core computes the full f4/f5 outputs for its batch (host keeps even cores').
#
# Per-core dataflow (all matmuls contract over the partition dim):
#   q,k   [128d, 2048]  = ReLU(W'.T @ x_t + b)      (BN folded host-side)
#   v0T   [m, 128d], v1T [n, 128d] tiles            (bias via K=1 ones-matmul)
#   E-str: QK n-block -> exp (ACT, accum=rowsum) -> E bf16 -> U1 += v1T.T @ E
#   F-str: QKT m-block -> exp (accum=colsum)     -> F bf16 -> U0 += v0T.T @ F
#   normalize U0/U1 by 1/rowsum, 1/colsum (reorder+recip+partition_broadcast)
#   AllGather pair -> add0/add1 full [256, 2048] -> f4/f5 as out-transposed
#   matmuls (lhsT=add, rhs=W.T) -> out0T/out1T [2048, 256] direct row DMA.

import numpy as np

B, N, M, C = 4, 2048, 2048, 256
H, Dh = 4, 64
EPS = 1e-5
NCORES = 8

_CACHE = {}


def _build_program():
    from contextlib import ExitStack

    import concourse.bass as bass
    import concourse.tile as tile
    from concourse import bacc, mybir
    from concourse.bass import ts

    f32 = mybir.dt.float32
    f32r = mybir.dt.float32r
    bf16 = mybir.dt.bfloat16
    AF = mybir.ActivationFunctionType
    ALU = mybir.AluOpType

    nc = bacc.Bacc(
        "TRN2",
        target_bir_lowering=False,
        debug=False,
        enable_asserts=False,
        num_devices=NCORES,
    )

    # ---- DRAM I/O ----
    x1t_d = nc.dram_tensor("x1t", [C, N], f32r, kind="ExternalInput").ap()
    x2t_d = nc.dram_tensor("x2t", [C, M], f32r, kind="ExternalInput").ap()
    wq_d = nc.dram_tensor("wq", [C, 128], f32r, kind="ExternalInput").ap()
    wk_d = nc.dram_tensor("wk", [C, 128], f32r, kind="ExternalInput").ap()
    wv0_d = nc.dram_tensor("wv0", [C, 128], f32r, kind="ExternalInput").ap()
    wv1_d = nc.dram_tensor("wv1", [C, 128], f32r, kind="ExternalInput").ap()
    bq_d = nc.dram_tensor("bq", [128, 1], f32, kind="ExternalInput").ap()
    bk_d = nc.dram_tensor("bk", [128, 1], f32, kind="ExternalInput").ap()
    bv0_d = nc.dram_tensor("bv0", [1, 128], f32r, kind="ExternalInput").ap()
    bv1_d = nc.dram_tensor("bv1", [1, 128], f32r, kind="ExternalInput").ap()
    w4t_d = nc.dram_tensor("w4t", [C, C], f32r, kind="ExternalInput").ap()
    w5t_d = nc.dram_tensor("w5t", [C, C], f32r, kind="ExternalInput").ap()
    b4_d = nc.dram_tensor("b4", [1, C], f32r, kind="ExternalInput").ap()
    b5_d = nc.dram_tensor("b5", [1, C], f32r, kind="ExternalInput").ap()
    ones_d = nc.dram_tensor("ones", [1, 128], f32r, kind="ExternalInput").ap()
    out0_d = nc.dram_tensor("out0t", [N, C], f32, kind="ExternalOutput").ap()
    out1_d = nc.dram_tensor("out1t", [M, C], f32, kind="ExternalOutput").ap()
    cc_in = nc.dram_tensor("cc_in", [256, N], f32r, kind="Internal").ap()
    cc_out = nc.dram_tensor("cc_out", [512, N], f32r, kind="Internal").ap()
    sc_d = nc.dram_tensor("sc", [4, 128, 16], f32, kind="Internal").ap()

    NB = N // 128  # 16 n-blocks
    MB = M // 128  # 16 m-blocks

    with tile.TileContext(nc) as tc, ExitStack() as ctx:
        const = ctx.enter_context(tc.tile_pool(name="const", bufs=1))
        # x1t/x2t (phase 1) and af0/af1 (phase 4+) share two slots via one tag
        xpool = ctx.enter_context(tc.tile_pool(name="xp", bufs=2))
        qkp = ctx.enter_context(tc.tile_pool(name="qkp", bufs=1))
        vp = ctx.enter_context(tc.tile_pool(name="vp", bufs=1))
        accp = ctx.enter_context(tc.tile_pool(name="accp", bufs=1))
        addp = ctx.enter_context(tc.tile_pool(name="addp", bufs=1))
        bcp = ctx.enter_context(tc.tile_pool(name="bcp", bufs=2))
        stream = ctx.enter_context(tc.tile_pool(name="stream", bufs=3))
        opool = ctx.enter_context(tc.tile_pool(name="opool", bufs=3))
        # PSUM: psS = 2 bufs x [128,1024] (2 banks each) ; psU = 1 x [128,2048]
        psS = ctx.enter_context(tc.tile_pool(name="psS", bufs=2, space="PSUM"))
        psU = ctx.enter_context(tc.tile_pool(name="psU", bufs=1, space="PSUM"))

        # ---- constants ----
        wq_sb = const.tile([128, 2, 128], f32r, tag="wq")
        wk_sb = const.tile([128, 2, 128], f32r, tag="wk")
        wv0_sb = const.tile([128, 2, 128], f32r, tag="wv0")
        wv1_sb = const.tile([128, 2, 128], f32r, tag="wv1")
        w4t_sb = const.tile([128, 2, 256], f32r, tag="w4t")
        w5t_sb = const.tile([128, 2, 256], f32r, tag="w5t")
        for dst, src in ((wq_sb, wq_d), (wk_sb, wk_d), (wv0_sb, wv0_d), (wv1_sb, wv1_d)):
            nc.sync.dma_start(dst[:], src.rearrange("(a p) d -> p a d", p=128))
        for dst, src in ((w4t_sb, w4t_d), (w5t_sb, w5t_d)):
            nc.sync.dma_start(dst[:], src.rearrange("(a p) d -> p a d", p=128))
        bq_sb = const.tile([128, 1], f32, tag="bq")
        bk_sb = const.tile([128, 1], f32, tag="bk")
        bv0_sb = const.tile([1, 128], f32r, tag="bv0")
        bv1_sb = const.tile([1, 128], f32r, tag="bv1")
        b4_sb = const.tile([1, 256], f32r, tag="b4")
        b5_sb = const.tile([1, 256], f32r, tag="b5")
        for dst, src in (
            (bq_sb, bq_d), (bk_sb, bk_d), (bv0_sb, bv0_d),
            (bv1_sb, bv1_d), (b4_sb, b4_d), (b5_sb, b5_d),
        ):
            nc.sync.dma_start(dst[:], src)
        ones_t = const.tile([1, 128], f32r, tag="ones")
        nc.sync.dma_start(ones_t[:], ones_d)

        x1t_sb = xpool.tile([128, 2, N], f32r, tag="xa")
        x2t_sb = xpool.tile([128, 2, M], f32r, tag="xa")
        nc.sync.dma_start(x1t_sb[:], x1t_d.rearrange("(a p) n -> p a n", p=128))
        nc.sync.dma_start(x2t_sb[:], x2t_d.rearrange("(a p) n -> p a n", p=128))

        # ---- phase 1: filters ----
        q_sb = qkp.tile([128, N], f32r, tag="q")
        k_sb = qkp.tile([128, M], f32r, tag="k")
        for dst, xt, w, bias in ((q_sb, x1t_sb, wq_sb, bq_sb), (k_sb, x2t_sb, wk_sb, bk_sb)):
            for j in range(4):  # 512-wide chunks
                ps = psS.tile([128, 1024], f32, tag="s")
                p5 = ps[:, 0:512]
                nc.tensor.matmul(
                    p5, w[:, 0],
                    xt[:, 0, ts(j, 512)], start=True, stop=False,
                )
                nc.tensor.matmul(
                    p5, w[:, 1],
                    xt[:, 1, ts(j, 512)], start=False, stop=True,
                )
                # ReLU(psum + bias) on DVE: (x + bias) then max 0
                nc.vector.tensor_scalar(
                    dst[:, ts(j, 512)], p5, bias[:], 0.0, op0=ALU.add, op1=ALU.max
                )

        v0t_sb = vp.tile([128, MB * 128], bf16, tag="v0t")  # [m-in-block, mb*128+d]
        v1t_sb = vp.tile([128, NB * 128], bf16, tag="v1t")
        for dst, xt, w, brow in (
            (v0t_sb, x2t_sb, wv0_sb, bv0_sb),
            (v1t_sb, x1t_sb, wv1_sb, bv1_sb),
        ):
            for mb in range(16):
                ps = psS.tile([128, 1024], f32, tag="s")
                p1 = ps[:, 0:128]
                nc.tensor.matmul(
                    p1, xt[:, 0, ts(mb, 128)],
                    w[:, 0], start=True, stop=False,
                )
                nc.tensor.matmul(
                    p1, xt[:, 1, ts(mb, 128)],
                    w[:, 1], start=False, stop=False,
                )
                nc.tensor.matmul(
                    p1, ones_t[:, 0:128],
                    brow[:], start=False, stop=True,
                )
                nc.vector.tensor_scalar_max(dst[:, ts(mb, 128)], p1, 0.0)

        # accumulator scratch for softmax sums: col index = h2*16 + blk
        rows_acc = [
            accp.tile([128, 32], f32, tag=f"ra{u}", name=f"rows_acc{u}")
            for u in range(2)
        ]
        cols_acc = [
            accp.tile([128, 32], f32, tag=f"ca{u}", name=f"cols_acc{u}")
            for u in range(2)
        ]

        # ---- phase 2: E-stream (QK -> exp -> U1 accumulate) ----
        U1ps = psU.tile([128, M], f32, tag="u")
        for nb in range(NB):
            for u in range(2):
                qs = q_sb[64 * u : 64 * u + 64, ts(nb, 128)]
                for h2 in range(2):
                    ps = psS.tile([128, 1024], f32, tag="s")
                    for j in range(2):
                        nc.tensor.matmul(
                            ps[:, ts(j, 512)], qs,
                            k_sb[64 * u : 64 * u + 64,
                                 h2 * 1024 + 512 * j : h2 * 1024 + 512 * (j + 1)
                                 ],
                            start=True, stop=True,
                        )
                    et = stream.tile([128, 1024], bf16, tag="st")
                    nc.scalar.activation(
                        et[:], ps[:], AF.Exp, scale=0.125,
                        accum_out=rows_acc[u][:, h2 * 16 + nb : h2 * 16 + nb + 1],
                    )
                    for j in range(2):
                        nc.tensor.matmul(
                            U1ps[64 * u : 64 * u + 64,
                                 h2 * 1024 + 512 * j : h2 * 1024 + 512 * (j + 1)],
                            v1t_sb[:, nb * 128 + 64 * u : nb * 128 + 64 * u + 64],
                            et[:, ts(j, 512)],
                            start=(nb == 0), stop=(nb == NB - 1),
                            tile_position=(0, 64 * u),
                        )
        # evict U1 unnormalized (colsum not known yet)
        add1_sb = addp.tile([128, M], f32r, tag="a1")
        nc.vector.tensor_copy(add1_sb[:], U1ps[:])

        # ---- phase 3: F-stream (QKT -> exp -> U0 accumulate) ----
        U0ps = psU.tile([128, N], f32, tag="u")
        for mb in range(MB):
            for u in range(2):
                ks = k_sb[64 * u : 64 * u + 64, ts(mb, 128)]
                for h2 in range(2):
                    ps = psS.tile([128, 1024], f32, tag="s")
                    for j in range(2):
                        nc.tensor.matmul(
                            ps[:, ts(j, 512)], ks,
                            q_sb[64 * u : 64 * u + 64,
                                 h2 * 1024 + 512 * j : h2 * 1024 + 512 * (j + 1)
                                 ],
                            start=True, stop=True,
                        )
                    ft = stream.tile([128, 1024], bf16, tag="st")
                    nc.scalar.activation(
                        ft[:], ps[:], AF.Exp, scale=0.125,
                        accum_out=cols_acc[u][:, h2 * 16 + mb : h2 * 16 + mb + 1],
                    )
                    for j in range(2):
                        nc.tensor.matmul(
                            U0ps[64 * u : 64 * u + 64,
                                 h2 * 1024 + 512 * j : h2 * 1024 + 512 * (j + 1)],
                            v0t_sb[:, mb * 128 + 64 * u : mb * 128 + 64 * u + 64],
                            ft[:, ts(j, 512)],
                            start=(mb == 0), stop=(mb == MB - 1),
                            tile_position=(0, 64 * u),
                        )

        # ---- softmax normalizers: [128,16] -> [1,2048] -> recip -> bcast ----
        add0_sb = addp.tile([128, N], f32r, tag="a0")
        for dirn, acc in ((0, rows_acc), (1, cols_acc)):
            for u in range(2):
                s16 = accp.tile([128, 16], f32, tag="s16")
                nc.vector.tensor_add(s16[:], acc[u][:, 0:16], acc[u][:, 16:32])
                flat = accp.tile([1, 2048], f32, tag="fl")
                # cross-partition reorder via DRAM: flat[0, 128*i + p] = s16[p, i]
                scs = sc_d[2 * dirn + u]
                nc.sync.dma_start(scs, s16[:])
                nc.sync.dma_start(flat[:], scs.rearrange("p i -> i p"))
                rec = accp.tile([1, 2048], f32, tag="rc")
                nc.vector.reciprocal(rec[:], flat[:])
                rbc = bcp.tile([128, 2048], f32, tag="bc")
                nc.gpsimd.partition_broadcast(rbc[:], rec[:])
                if dirn == 0:
                    # fused normalize-evict of U0
                    nc.vector.tensor_mul(
                        add0_sb[64 * u : 64 * u + 64, :],
                        U0ps[64 * u : 64 * u + 64, :],
                        rbc[64 * u : 64 * u + 64, :],
                    )
                else:
                    nc.vector.tensor_mul(
                        add1_sb[64 * u : 64 * u + 64, :],
                        add1_sb[64 * u : 64 * u + 64, :],
                        rbc[64 * u : 64 * u + 64, :],
                    )

        # ---- phase 4: pair AllGather of add0/add1 ----
        nc.sync.dma_start(cc_in[0:128, :], add0_sb[:])
        nc.sync.dma_start(cc_in[128:256, :], add1_sb[:])
        nc.gpsimd.collective_compute(
            "AllGather",
            ALU.bypass,
            replica_groups=[[0, 1], [2, 3], [4, 5], [6, 7]],
            ins=[cc_in],
            outs=[cc_out],
        )
        af0 = xpool.tile([128, 2, N], f32r, tag="xa")
        af1 = xpool.tile([128, 2, M], f32r, tag="xa")
        nc.sync.dma_start(af0[:, 0, :], cc_out[0:128, :])
        nc.sync.dma_start(af0[:, 1, :], cc_out[256:384, :])
        nc.sync.dma_start(af1[:, 0, :], cc_out[128:256, :])
        nc.sync.dma_start(af1[:, 1, :], cc_out[384:512, :])

        # ---- phase 5: output filters, out-transposed ----
        for out_d, af, wt, brow in (
            (out0_d, af0, w4t_sb, b4_sb),
            (out1_d, af1, w5t_sb, b5_sb),
        ):
            for nb in range(NB):
                ps = psS.tile([128, 1024], f32, tag="s")
                p2 = ps[:, 0:256]
                nc.tensor.matmul(
                    p2, af[:, 0, ts(nb, 128)],
                    wt[:, 0], start=True, stop=False,
                )
                nc.tensor.matmul(
                    p2, af[:, 1, ts(nb, 128)],
                    wt[:, 1], start=False, stop=False,
                )
                nc.tensor.matmul(
                    p2, ones_t[:, 0:128],
                    brow[:], start=False, stop=True,
                )
                ot = opool.tile([128, 256], f32, tag="ot")
                nc.vector.tensor_scalar_max(ot[:], p2, 0.0)
                nc.sync.dma_start(out_d[ts(nb, 128), :], ot[:])

    nc.compile()
    return nc


def _prep_core_inputs(inputs):
    """Fold BN into weights, build per-core input maps."""
    x1 = np.ascontiguousarray(inputs["x1"], dtype=np.float32)
    x2 = np.ascontiguousarray(inputs["x2"], dtype=np.float32)
    Ws = np.asarray(inputs["Ws"], dtype=np.float32)
    bs = np.asarray(inputs["bs"], dtype=np.float32)
    g = np.asarray(inputs["gammas"], dtype=np.float32)
    be = np.asarray(inputs["betas"], dtype=np.float32)
    mn = np.asarray(inputs["means"], dtype=np.float32)
    vr = np.asarray(inputs["vars_"], dtype=np.float32)

    s = g / np.sqrt(vr + EPS)  # [6, C]
    Wf = Ws * s[:, :, None]  # rows scaled
    bf = s * (bs - mn) + be

    WfT = np.ascontiguousarray(np.swapaxes(Wf, 1, 2))  # [6, C(in), C(out)]
    x1t = np.ascontiguousarray(np.swapaxes(x1, 1, 2))  # [B, C, N]
    x2t = np.ascontiguousarray(np.swapaxes(x2, 1, 2))

    in_maps = []
    for core in range(NCORES):
        b, par = core // 2, core % 2
        sl = slice(par * 128, par * 128 + 128)
        in_maps.append(
            {
                "x1t": x1t[b],
                "x2t": x2t[b],
                "wq": np.ascontiguousarray(WfT[0][:, sl]),
                "wk": np.ascontiguousarray(WfT[1][:, sl]),
                "wv0": np.ascontiguousarray(WfT[2][:, sl]),
                "wv1": np.ascontiguousarray(WfT[3][:, sl]),
                "bq": np.ascontiguousarray(bf[0][sl]).reshape(128, 1),
                "bk": np.ascontiguousarray(bf[1][sl]).reshape(128, 1),
                "bv0": np.ascontiguousarray(bf[2][sl]).reshape(1, 128),
                "bv1": np.ascontiguousarray(bf[3][sl]).reshape(1, 128),
                "w4t": WfT[4],
                "w5t": WfT[5],
                "b4": bf[4].reshape(1, C),
                "b5": bf[5].reshape(1, C),
                "ones": np.ones((1, 128), np.float32),
            }
        )
    return in_maps


def kernel(**inputs):
    from concourse import bass_utils

    if "nc" not in _CACHE:
        _CACHE["nc"] = _build_program()
    nc = _CACHE["nc"]

    in_maps = _prep_core_inputs(inputs)
    res = bass_utils.run_bass_kernel_spmd(
        nc, in_maps, core_ids=list(range(NCORES))
    )
    results = res.results
    out0 = np.stack([results[2 * b]["out0t"] for b in range(B)])
    out1 = np.stack([results[2 * b]["out1t"] for b in range(B)])
    return out0, out1


# revision 10
# speedup vs baseline: 1.2709x; 1.2709x over previous
# Trainium2 Bass kernel for nn_AttentionPropagation (SuperGlue-style bidirectional
# attentional propagation): 6x (1x1conv+BN+ReLU) filters + QK attention with
# softmax over BOTH axes + two aggregations + output filters.
#
# Sharding: 16 (batch, head) units over 8 cores -> each core owns batch b=core//2
# and a contiguous 128-channel (2-head) slice of the filter outputs.  The final
# filters f4/f5 need all 4 heads of a batch, so core pairs {2b, 2b+1} exchange
# their normalized 128-channel halves of add0/add1 with an AllGather, then each
# core computes the full f4/f5 outputs for its batch (host keeps even cores').
#
# Per-core dataflow (all matmuls contract over the partition dim):
#   q,k   [128d, 2048]  = ReLU(W'.T @ x_t + b)      (BN folded host-side)
#   v0T   [m, 128d], v1T [n, 128d] tiles            (bias via K=1 ones-matmul)
#   E-str: QK n-block -> exp (ACT, accum=rowsum) -> E bf16 -> U1 += v1T.T @ E
#   F-str: QKT m-block -> exp (accum=colsum)     -> F bf16 -> U0 += v0T.T @ F
#   normalize U0/U1 by 1/rowsum, 1/colsum (reorder+recip+partition_broadcast)
#   AllGather pair -> add0/add1 full [256, 2048] -> f4/f5 as out-transposed
#   matmuls (lhsT=add, rhs=W.T) -> out0T/out1T [2048, 256] direct row DMA.

import numpy as np

B, N, M, C = 4, 2048, 2048, 256
H, Dh = 4, 64
EPS = 1e-5
NCORES = 8

_CACHE = {}


def _build_program():
    from contextlib import ExitStack

    import concourse.bass as bass
    import concourse.tile as tile
    from concourse import bacc, mybir
    from concourse.bass import ts

    f32 = mybir.dt.float32
    f32r = mybir.dt.float32r
    bf16 = mybir.dt.bfloat16
    AF = mybir.ActivationFunctionType
    ALU = mybir.AluOpType

    nc = bacc.Bacc(
        "TRN2",
        target_bir_lowering=False,
        debug=False,
        enable_asserts=False,
        num_devices=NCORES,
    )

    # ---- DRAM I/O ----
    x1t_d = nc.dram_tensor("x1t", [C, N], bf16, kind="ExternalInput").ap()
    x2t_d = nc.dram_tensor("x2t", [C, M], bf16, kind="ExternalInput").ap()
    wq_d = nc.dram_tensor("wq", [C, 128], bf16, kind="ExternalInput").ap()
    wk_d = nc.dram_tensor("wk", [C, 128], bf16, kind="ExternalInput").ap()
    wv0_d = nc.dram_tensor("wv0", [C, 128], bf16, kind="ExternalInput").ap()
    wv1_d = nc.dram_tensor("wv1", [C, 128], bf16, kind="ExternalInput").ap()
    bq_d = nc.dram_tensor("bq", [128, 1], f32, kind="ExternalInput").ap()
    bk_d = nc.dram_tensor("bk", [128, 1], f32, kind="ExternalInput").ap()
    bv0_d = nc.dram_tensor("bv0", [1, 128], bf16, kind="ExternalInput").ap()
    bv1_d = nc.dram_tensor("bv1", [1, 128], bf16, kind="ExternalInput").ap()
    w4t_d = nc.dram_tensor("w4t", [C, C], bf16, kind="ExternalInput").ap()
    w5t_d = nc.dram_tensor("w5t", [C, C], bf16, kind="ExternalInput").ap()
    b4_d = nc.dram_tensor("b4", [1, C], bf16, kind="ExternalInput").ap()
    b5_d = nc.dram_tensor("b5", [1, C], bf16, kind="ExternalInput").ap()
    ones_d = nc.dram_tensor("ones", [1, 128], bf16, kind="ExternalInput").ap()
    out0_d = nc.dram_tensor("out0t", [N, C], f32, kind="ExternalOutput").ap()
    out1_d = nc.dram_tensor("out1t", [M, C], f32, kind="ExternalOutput").ap()
    cc_in = nc.dram_tensor("cc_in", [256, N], bf16, kind="Internal").ap()
    cc_out = nc.dram_tensor("cc_out", [512, N], bf16, kind="Internal").ap()
    sc_d = nc.dram_tensor("sc", [4, 128, 16], f32, kind="Internal").ap()

    NB = N // 128  # 16 n-blocks
    MB = M // 128  # 16 m-blocks

    with tile.TileContext(nc) as tc, ExitStack() as ctx:
        const = ctx.enter_context(tc.tile_pool(name="const", bufs=1))
        # x1t/x2t (phase 1) and af0/af1 (phase 4+) share two slots via one tag
        xpool = ctx.enter_context(tc.tile_pool(name="xp", bufs=2))
        qkp = ctx.enter_context(tc.tile_pool(name="qkp", bufs=1))
        vp = ctx.enter_context(tc.tile_pool(name="vp", bufs=1))
        accp = ctx.enter_context(tc.tile_pool(name="accp", bufs=1))
        addp = ctx.enter_context(tc.tile_pool(name="addp", bufs=1))
        bcp = ctx.enter_context(tc.tile_pool(name="bcp", bufs=1))
        stream = ctx.enter_context(tc.tile_pool(name="stream", bufs=4))
        opool = ctx.enter_context(tc.tile_pool(name="opool", bufs=3))
        # PSUM: psS = 2 bufs x [128,1024] (2 banks each) ; psU = 1 x [128,2048]
        psS = ctx.enter_context(tc.tile_pool(name="psS", bufs=2, space="PSUM"))
        psU = ctx.enter_context(tc.tile_pool(name="psU", bufs=1, space="PSUM"))

        # ---- constants ----
        wq_sb = const.tile([128, 2, 128], bf16, tag="wq")
        wk_sb = const.tile([128, 2, 128], bf16, tag="wk")
        wv0_sb = const.tile([128, 2, 128], bf16, tag="wv0")
        wv1_sb = const.tile([128, 2, 128], bf16, tag="wv1")
        w4t_sb = const.tile([128, 2, 256], bf16, tag="w4t")
        w5t_sb = const.tile([128, 2, 256], bf16, tag="w5t")
        for dst, src in ((wq_sb, wq_d), (wk_sb, wk_d), (wv0_sb, wv0_d), (wv1_sb, wv1_d)):
            nc.sync.dma_start(dst[:], src.rearrange("(a p) d -> p a d", p=128))
        for dst, src in ((w4t_sb, w4t_d), (w5t_sb, w5t_d)):
            nc.sync.dma_start(dst[:], src.rearrange("(a p) d -> p a d", p=128))
        bq_sb = const.tile([128, 1], f32, tag="bq")
        bk_sb = const.tile([128, 1], f32, tag="bk")
        bv0_sb = const.tile([1, 128], bf16, tag="bv0")
        bv1_sb = const.tile([1, 128], bf16, tag="bv1")
        b4_sb = const.tile([1, 256], bf16, tag="b4")
        b5_sb = const.tile([1, 256], bf16, tag="b5")
        for dst, src in (
            (bq_sb, bq_d), (bk_sb, bk_d), (bv0_sb, bv0_d),
            (bv1_sb, bv1_d), (b4_sb, b4_d), (b5_sb, b5_d),
        ):
            nc.sync.dma_start(dst[:], src)
        ones_t = const.tile([1, 128], bf16, tag="ones")
        nc.sync.dma_start(ones_t[:], ones_d)

        x1t_sb = xpool.tile([128, 2, N], bf16, tag="xa")
        x2t_sb = xpool.tile([128, 2, M], bf16, tag="xa")
        nc.sync.dma_start(x1t_sb[:], x1t_d.rearrange("(a p) n -> p a n", p=128))
        nc.sync.dma_start(x2t_sb[:], x2t_d.rearrange("(a p) n -> p a n", p=128))

        # ---- phase 1: filters ----
        q_sb = qkp.tile([128, N], bf16, tag="q")
        k_sb = qkp.tile([128, M], bf16, tag="k")
        for dst, xt, w, bias in ((q_sb, x1t_sb, wq_sb, bq_sb), (k_sb, x2t_sb, wk_sb, bk_sb)):
            for j in range(4):  # 512-wide chunks
                ps = psS.tile([128, 1024], f32, tag="s")
                p5 = ps[:, 0:512]
                nc.tensor.matmul(
                    p5, w[:, 0],
                    xt[:, 0, ts(j, 512)], start=True, stop=False,
                )
                nc.tensor.matmul(
                    p5, w[:, 1],
                    xt[:, 1, ts(j, 512)], start=False, stop=True,
                )
                # ReLU(psum + bias) on DVE: (x + bias) then max 0
                nc.vector.tensor_scalar(
                    dst[:, ts(j, 512)], p5, bias[:], 0.0, op0=ALU.add, op1=ALU.max
                )

        v0t_sb = vp.tile([128, MB * 128], bf16, tag="v0t")  # [m-in-block, mb*128+d]
        v1t_sb = vp.tile([128, NB * 128], bf16, tag="v1t")
        for dst, xt, w, brow in (
            (v0t_sb, x2t_sb, wv0_sb, bv0_sb),
            (v1t_sb, x1t_sb, wv1_sb, bv1_sb),
        ):
            for mb in range(16):
                ps = psS.tile([128, 1024], f32, tag="s")
                p1 = ps[:, 0:128]
                nc.tensor.matmul(
                    p1, xt[:, 0, ts(mb, 128)],
                    w[:, 0], start=True, stop=False,
                )
                nc.tensor.matmul(
                    p1, xt[:, 1, ts(mb, 128)],
                    w[:, 1], start=False, stop=False,
                )
                nc.tensor.matmul(
                    p1, ones_t[:, 0:128],
                    brow[:], start=False, stop=True,
                )
                nc.vector.tensor_scalar_max(dst[:, ts(mb, 128)], p1, 0.0)

        # accumulator scratch for softmax sums: col index = h2*16 + blk
        rows_acc = [
            accp.tile([128, 32], f32, tag=f"ra{u}", name=f"rows_acc{u}")
            for u in range(2)
        ]
        cols_acc = [
            accp.tile([128, 32], f32, tag=f"ca{u}", name=f"cols_acc{u}")
            for u in range(2)
        ]

        # ---- phase 2: E-stream (QK -> exp -> U1 accumulate) ----
        U1ps = psU.tile([128, M], f32, tag="u")
        for nb in range(NB):
            for u in range(2):
                qs = q_sb[64 * u : 64 * u + 64, ts(nb, 128)]
                for h2 in range(2):
                    ps = psS.tile([128, 1024], f32, tag="s")
                    for j in range(2):
                        nc.tensor.matmul(
                            ps[:, ts(j, 512)], qs,
                            k_sb[64 * u : 64 * u + 64,
                                 h2 * 1024 + 512 * j : h2 * 1024 + 512 * (j + 1)
                                 ],
                            start=True, stop=True,
                        )
                    et = stream.tile([128, 1024], bf16, tag="st")
                    nc.scalar.activation(
                        et[:], ps[:], AF.Exp, scale=0.125,
                        accum_out=rows_acc[u][:, h2 * 16 + nb : h2 * 16 + nb + 1],
                    )
                    for j in range(2):
                        nc.tensor.matmul(
                            U1ps[64 * u : 64 * u + 64,
                                 h2 * 1024 + 512 * j : h2 * 1024 + 512 * (j + 1)],
                            v1t_sb[:, nb * 128 + 64 * u : nb * 128 + 64 * u + 64],
                            et[:, ts(j, 512)],
                            start=(nb == 0), stop=(nb == NB - 1),
                            tile_position=(0, 64 * u),
                        )
        # evict U1 unnormalized (colsum not known yet)
        add1_sb = addp.tile([128, M], bf16, tag="a1")
        nc.vector.tensor_copy(add1_sb[:], U1ps[:])

        # ---- phase 3: F-stream (QKT -> exp -> U0 accumulate) ----
        U0ps = psU.tile([128, N], f32, tag="u")
        for mb in range(MB):
            for u in range(2):
                ks = k_sb[64 * u : 64 * u + 64, ts(mb, 128)]
                for h2 in range(2):
                    ps = psS.tile([128, 1024], f32, tag="s")
                    for j in range(2):
                        nc.tensor.matmul(
                            ps[:, ts(j, 512)], ks,
                            q_sb[64 * u : 64 * u + 64,
                                 h2 * 1024 + 512 * j : h2 * 1024 + 512 * (j + 1)
                                 ],
                            start=True, stop=True,
                        )
                    ft = stream.tile([128, 1024], bf16, tag="st")
                    nc.scalar.activation(
                        ft[:], ps[:], AF.Exp, scale=0.125,
                        accum_out=cols_acc[u][:, h2 * 16 + mb : h2 * 16 + mb + 1],
                    )
                    for j in range(2):
                        nc.tensor.matmul(
                            U0ps[64 * u : 64 * u + 64,
                                 h2 * 1024 + 512 * j : h2 * 1024 + 512 * (j + 1)],
                            v0t_sb[:, mb * 128 + 64 * u : mb * 128 + 64 * u + 64],
                            ft[:, ts(j, 512)],
                            start=(mb == 0), stop=(mb == MB - 1),
                            tile_position=(0, 64 * u),
                        )

        # ---- softmax normalizers: recip [128,16] -> reorder -> bcast ----
        add0_sb = addp.tile([128, N], bf16, tag="a0")

        def norm_chain(dirn, u, acc):
            s16 = accp.tile([128, 16], f32, tag=f"s16_{dirn}{u}", name=f"s16_{dirn}{u}")
            nc.vector.tensor_add(s16[:], acc[u][:, 0:16], acc[u][:, 16:32])
            rec16 = accp.tile([128, 16], f32, tag=f"r16_{dirn}{u}", name=f"r16_{dirn}{u}")
            nc.vector.reciprocal(rec16[:], s16[:])
            # cross-partition reorder via DRAM: flat[0, 128*i + p] = rec16[p, i]
            scs = sc_d[2 * dirn + u]
            nc.sync.dma_start(scs, rec16[:])
            flat = accp.tile([1, 2048], f32, tag=f"fl_{dirn}{u}", name=f"fl_{dirn}{u}")
            nc.sync.dma_start(flat[:], scs.rearrange("p i -> i p"))
            rbc = bcp.tile([128, 2048], f32, tag=f"bc_{dirn}{u}", name=f"bc_{dirn}{u}")
            nc.gpsimd.partition_broadcast(rbc[:], flat[:])
            return rbc

        for u in range(2):
            rbc = norm_chain(1, u, cols_acc)
            nc.vector.tensor_mul(
                add1_sb[64 * u : 64 * u + 64, :],
                add1_sb[64 * u : 64 * u + 64, :],
                rbc[64 * u : 64 * u + 64, :],
            )
        for u in range(2):
            rbc = norm_chain(0, u, rows_acc)
            nc.vector.tensor_mul(
                add0_sb[64 * u : 64 * u + 64, :],
                U0ps[64 * u : 64 * u + 64, :],
                rbc[64 * u : 64 * u + 64, :],
            )

        # ---- phase 4: pair AllGather of add0/add1 ----
        nc.sync.dma_start(cc_in[0:128, :], add0_sb[:])
        nc.sync.dma_start(cc_in[128:256, :], add1_sb[:])
        nc.gpsimd.collective_compute(
            "AllGather",
            ALU.bypass,
            replica_groups=[[0, 1], [2, 3], [4, 5], [6, 7]],
            ins=[cc_in],
            outs=[cc_out],
        )
        af0 = xpool.tile([128, 2, N], bf16, tag="xa")
        af1 = xpool.tile([128, 2, M], bf16, tag="xa")
        nc.sync.dma_start(af0[:, 0, :], cc_out[0:128, :])
        nc.sync.dma_start(af0[:, 1, :], cc_out[256:384, :])
        nc.sync.dma_start(af1[:, 0, :], cc_out[128:256, :])
        nc.sync.dma_start(af1[:, 1, :], cc_out[384:512, :])

        # ---- phase 5: output filters, out-transposed ----
        for out_d, af, wt, brow in (
            (out0_d, af0, w4t_sb, b4_sb),
            (out1_d, af1, w5t_sb, b5_sb),
        ):
            for nb in range(NB):
                ps = psS.tile([128, 1024], f32, tag="s")
                p2 = ps[:, 0:256]
                nc.tensor.matmul(
                    p2, af[:, 0, ts(nb, 128)],
                    wt[:, 0], start=True, stop=False,
                )
                nc.tensor.matmul(
                    p2, af[:, 1, ts(nb, 128)],
                    wt[:, 1], start=False, stop=False,
                )
                nc.tensor.matmul(
                    p2, ones_t[:, 0:128],
                    brow[:], start=False, stop=True,
                )
                ot = opool.tile([128, 256], f32, tag="ot")
                nc.vector.tensor_scalar_max(ot[:], p2, 0.0)
                nc.sync.dma_start(out_d[ts(nb, 128), :], ot[:])

    nc.compile()
    return nc


def _prep_core_inputs(inputs):
    """Fold BN into weights, build per-core input maps."""
    x1 = np.ascontiguousarray(inputs["x1"], dtype=np.float32)
    x2 = np.ascontiguousarray(inputs["x2"], dtype=np.float32)
    Ws = np.asarray(inputs["Ws"], dtype=np.float32)
    bs = np.asarray(inputs["bs"], dtype=np.float32)
    g = np.asarray(inputs["gammas"], dtype=np.float32)
    be = np.asarray(inputs["betas"], dtype=np.float32)
    mn = np.asarray(inputs["means"], dtype=np.float32)
    vr = np.asarray(inputs["vars_"], dtype=np.float32)

    s = g / np.sqrt(vr + EPS)  # [6, C]
    Wf = Ws * s[:, :, None]  # rows scaled
    bf = s * (bs - mn) + be

    import ml_dtypes

    bfl = ml_dtypes.bfloat16
    WfT = np.ascontiguousarray(np.swapaxes(Wf, 1, 2)).astype(bfl)  # [6, C, C]
    x1t = np.ascontiguousarray(np.swapaxes(x1, 1, 2)).astype(bfl)  # [B, C, N]
    x2t = np.ascontiguousarray(np.swapaxes(x2, 1, 2)).astype(bfl)
    bfb = bf.astype(bfl)

    in_maps = []
    for core in range(NCORES):
        b, par = core // 2, core % 2
        sl = slice(par * 128, par * 128 + 128)
        in_maps.append(
            {
                "x1t": x1t[b],
                "x2t": x2t[b],
                "wq": np.ascontiguousarray(WfT[0][:, sl]),
                "wk": np.ascontiguousarray(WfT[1][:, sl]),
                "wv0": np.ascontiguousarray(WfT[2][:, sl]),
                "wv1": np.ascontiguousarray(WfT[3][:, sl]),
                "bq": np.ascontiguousarray(bf[0][sl]).reshape(128, 1),
                "bk": np.ascontiguousarray(bf[1][sl]).reshape(128, 1),
                "bv0": np.ascontiguousarray(bfb[2][sl]).reshape(1, 128),
                "bv1": np.ascontiguousarray(bfb[3][sl]).reshape(1, 128),
                "w4t": WfT[4],
                "w5t": WfT[5],
                "b4": bfb[4].reshape(1, C),
                "b5": bfb[5].reshape(1, C),
                "ones": np.ones((1, 128), bfl),
            }
        )
    return in_maps


def kernel(**inputs):
    from concourse import bass_utils

    if "nc" not in _CACHE:
        _CACHE["nc"] = _build_program()
    nc = _CACHE["nc"]

    in_maps = _prep_core_inputs(inputs)
    res = bass_utils.run_bass_kernel_spmd(
        nc, in_maps, core_ids=list(range(NCORES))
    )
    results = res.results
    out0 = np.stack([results[2 * b]["out0t"] for b in range(B)])
    out1 = np.stack([results[2 * b]["out1t"] for b in range(B)])
    return out0, out1
